# revision 27
# baseline (speedup 1.0000x reference)
"""Multi-head causal attention (B=2, T=2048, D=1024, H=16) on 8 trn2 cores.

Sharding: core c = (batch b, head-group g) with b = c//4, g = c%4.
Each core computes Q/K/V projections for its 4 heads (256 features),
causal attention, and its partial output projection; the host sums the
4 per-batch partials (the w_o all-reduce) and stacks batches.

Device schedule (per core):
 - q is processed in four 512-wide stripes; per stripe, each head PAIR
   (sharing an eb block, rows 0:64 / 64:128) runs its score matmuls
   concurrently in disjoint PE row groups (K=64 each).
 - exp runs pair-wide on [128, 2, 512] PSUM tiles (one ACTIVATE per
   key chunk), causal masking via one gpsimd affine_select on the
   diagonal chunks.
 - softmax denominator falls out of the PV matmul via a ones column in
   the V stationary (M=65); normalization = reciprocal_approx_fast on
   the denominator row + gpsimd partition_broadcast + one fused
   multiply+bf16-cast tensor_tensor.
 - projections for stripe s+1 and the output projection of stripe s are
   issued after attention of stripe s so the Tile scheduler fills PE
   gaps during ACT-bound attention chunks.
"""

import math

import ml_dtypes
import numpy as np

BF16NP = ml_dtypes.bfloat16

import concourse.bass as bass
from concourse import bacc
import concourse.mybir as mybir
import concourse.tile as tile
from concourse.bass_utils import run_bass_kernel_spmd

F32 = mybir.dt.float32
AF = mybir.ActivationFunctionType
ALU = mybir.AluOpType
BF16 = mybir.dt.bfloat16
F8 = mybir.dt.float8e4
F8NP = ml_dtypes.float8_e4m3

B, T, D, H = 2, 2048, 1024, 16
NCORES = 8
G = 4             # head groups (tensor parallel); cores = B * G
HPG = H // G      # 4 heads per core
DH = D // H       # 64 head dim
E = D // G        # 256 features per core
EB = E // 128     # 2 e-blocks of 128
KD = D // 128     # 8 contraction chunks for projections
SN = T // 512     # 4 q stripes of 512
TC = T // 128     # 16 128-wide key chunks


def build_nc():
    nc = bacc.Bacc(None)
    xqT = nc.declare_dram_parameter("xqT", [D, T], BF16, isOutput=False)
    xkT = nc.declare_dram_parameter("xkT", [D, T], BF16, isOutput=False)
    xvT = nc.declare_dram_parameter("xvT", [D, T], BF16, isOutput=False)
    wqT = nc.declare_dram_parameter("wqT", [D, E], BF16, isOutput=False)
    wkT = nc.declare_dram_parameter("wkT", [D, E], BF16, isOutput=False)
    wvT = nc.declare_dram_parameter("wvT", [D, E], BF16, isOutput=False)
    woT = nc.declare_dram_parameter("woT", [E, D], BF16, isOutput=False)
    outp = nc.declare_dram_parameter("outp", [T, D], BF16, isOutput=True)

    with tile.TileContext(nc) as tc:
        with (
            tc.tile_pool(name="persist", bufs=1) as persist,
            tc.tile_pool(name="xs", bufs=12) as xs,
            tc.tile_pool(name="pt", bufs=6) as ptp,
            tc.tile_pool(name="rr", bufs=2) as rrp,
            tc.tile_pool(name="rb", bufs=2) as rbp,
            tc.tile_pool(name="odd", bufs=2) as oddp,
            tc.tile_pool(name="outs", bufs=3) as outsp,
            tc.tile_pool(name="ps", bufs=2, space="PSUM") as psp,
            tc.tile_pool(name="po", bufs=2, space="PSUM") as pop,
            tc.tile_pool(name="aux", bufs=2, space="PSUM") as auxp,
        ):
            wq_sb = persist.tile([128, KD, E], BF16, tag="wq")
            wk_sb = persist.tile([128, KD, E], BF16, tag="wk")
            wv_sb = persist.tile([128, KD, E], BF16, tag="wv")
            wo_sb = persist.tile([128, EB, D], BF16, tag="wo")
            QT = persist.tile([128, EB, T], BF16, tag="QT")
            KT = persist.tile([128, EB, T], BF16, tag="KT")
            Vp = persist.tile([128, TC, HPG, DH + 1], BF16, tag="Vp")
            ONpk = persist.tile([128, EB, T], BF16, tag="ONpk")

            # weights split into per-kd-pair chunks so the first projection
            # matmuls are not gated on a single monolithic queue transfer;
            # descriptor issue alternates between the two HWDGE queue
            # engines (sync, scalar) to halve queue-op serialization
            wqr = wqT[:, :].rearrange("(c p) e -> p c e", p=128)
            wkr = wkT[:, :].rearrange("(c p) e -> p c e", p=128)
            wvr = wvT[:, :].rearrange("(c p) e -> p c e", p=128)
            for j in range(4):
                nc.sync.dma_start(wq_sb[:, 2 * j : 2 * j + 2, :], wqr[:, 2 * j : 2 * j + 2, :])
                nc.sync.dma_start(wk_sb[:, 2 * j : 2 * j + 2, :], wkr[:, 2 * j : 2 * j + 2, :])

            # ones column of Vp: P @ [V | 1] makes the softmax denominator
            # fall out of the PV matmul as psum row 64
            nc.vector.memset(Vp[:, :, :, DH : DH + 1], 1.0)

            # causal masks for the 4 diagonal sub-chunks, built once:
            # mask[m][p, j] = 1 if j >= p + 128*m else 0
            masks = []
            for m in range(4):
                mt = persist.tile([128, 512], BF16, tag=f"mask{m}", name=f"mask{m}")
                nc.vector.memset(mt[:], 1.0)
                nc.gpsimd.affine_select(
                    out=mt[:],
                    in_=mt[:],
                    pattern=[[1, 512]],
                    compare_op=ALU.is_ge,
                    fill=0.0,
                    base=-(128 * m),
                    channel_multiplier=-1,
                )
                masks.append(mt)

            # prefetch all x stripes up front, stripe-major so stripe 0 of
            # q/k/v lands first (12 MB total; DMA overlaps compute)
            xtiles = {}
            for s in range(SN):
                for name, xdram in (("q", xqT), ("k", xkT), ("v", xvT)):
                    xr = xdram[:, :].rearrange("(c p) t -> p c t", p=128)
                    t = xs.tile([128, KD, 512], BF16, tag="x", name=f"x_{name}{s}")
                    nj = 8 if s == 0 else 4
                    for j in range(nj):
                        w = KD // nj
                        nc.sync.dma_start(
                            t[:, w * j : w * j + w, :],
                            xr[:, w * j : w * j + w, 512 * s : 512 * s + 512],
                        )
                    xtiles[(name, s)] = t
                if s == 0:
                    for j in range(4):
                        nc.sync.dma_start(
                            wv_sb[:, 2 * j : 2 * j + 2, :],
                            wvr[:, 2 * j : 2 * j + 2, :],
                        )
                if s == 1:
                    nc.sync.dma_start(
                        wo_sb[:], woT[:, :].rearrange("(c p) d -> p c d", p=128)
                    )

            def qk_piece(s, eb, which):
                # Q (which=0) or K (which=1): dest[e, t] = w[d, e].T @ x[d, t]
                wsb, xn, dest = (
                    (wq_sb, "q", QT) if which == 0 else (wk_sb, "k", KT)
                )
                xt = xtiles[(xn, s)]
                acc = auxp.tile([128, 512], F32, tag="aux", name=f"qk{s}{eb}{xn}")
                for kd in range(KD):
                    nc.tensor.matmul(
                        acc[:],
                        wsb[:, kd, 128 * eb : 128 * eb + 128],
                        xt[:, kd, :],
                        start=(kd == 0),
                        stop=(kd == KD - 1),
                    )
                nc.vector.tensor_copy(
                    dest[:, eb, 512 * s : 512 * s + 512], acc[:]
                )

            def qk_proj(s, eb):
                qk_piece(s, eb, 0)
                qk_piece(s, eb, 1)

            def v_proj_sub(s, sub):
                # V, natural layout: V[t, e] = x[d, t].T @ w[d, e]
                xt = xtiles[("v", s)]
                acc = auxp.tile([128, E], F32, tag="aux", name=f"v{s}{sub}")
                for kd in range(KD):
                    nc.tensor.matmul(
                        acc[:],
                        xt[:, kd, 128 * sub : 128 * sub + 128],
                        wv_sb[:, kd, :],
                        start=(kd == 0),
                        stop=(kd == KD - 1),
                    )
                nc.vector.tensor_copy(
                    Vp[:, 4 * s + sub, :, 0:DH],
                    acc[:].rearrange("p (h d) -> p h d", h=HPG),
                )

            def proj_stripe(s):
                # pair-0 dependencies first so attention can start sooner
                qk_proj(s, 0)
                for sub in range(4):
                    v_proj_sub(s, sub)
                qk_proj(s, 1)

            def att_stripe(p, s, inject=None):
                eb = p
                q0 = 512 * s
                kc_hi = 4 * (s + 1)
                pOs = [
                    pop.tile([128, 512], F32, tag="po", name=f"pO{p}_{s}_{i}")
                    for i in range(2)
                ]
                for hh in range(2):
                    # rows 65-95 feed stream_shuffle; only row 64 is real
                    # (PV overwrites it). 32-aligned partition base required.
                    nc.vector.memset(pOs[hh][64:96, :], 0.0)
                # software pipeline: PV(kc-1) is issued AFTER scores+exp(kc)
                # so the in-order PE queue never head-of-line blocks on exp
                pes = {}

                def pv_chunk(kc):
                    pe = pes.pop(kc)
                    for hh in range(2):
                        nc.tensor.matmul(
                            pOs[hh][0:65, :],
                            Vp[:, kc, 2 * p + hh, :],
                            pe[:, hh, :],
                            start=(kc == 0),
                            stop=(kc == kc_hi - 1),
                        )

                for kc in range(kc_hi):
                    if kc == 2 and inject:
                        # K/V projections for THIS stripe: only the diagonal
                        # chunks (kc >= 4s) need them, so they slot in here
                        # while attention still has pre-diagonal runway
                        while inject:
                            inject.pop(0)()
                    pS = psp.tile([128, 2, 512], F32, tag="ps")
                    for hh in range(2):
                        r0 = 64 * hh
                        nc.tensor.matmul(
                            pS[:, hh, :],
                            KT[r0 : r0 + 64, eb, 128 * kc : 128 * kc + 128],
                            QT[r0 : r0 + 64, eb, q0 : q0 + 512],
                            start=True,
                            stop=True,
                        )
                    pe = ptp.tile([128, 2, 512], BF16, tag="pt")
                    nc.scalar.activation(pe[:], pS[:], AF.Exp)
                    if kc // 4 == s:
                        # causal mask on the diagonal chunk, both heads:
                        # keep q >= key_row + 128*(kc%4). bf16 multiply by
                        # the precomputed 0/1 mask on the (lighter) DVE
                        mt = masks[kc % 4]
                        nc.vector.tensor_tensor(
                            out=pe[:],
                            in0=pe[:],
                            in1=mt[:, None, :].broadcast_to([128, 2, 512]),
                            op=ALU.mult,
                        )
                    pes[kc] = pe
                    if kc >= 1:
                        pv_chunk(kc - 1)
                pv_chunk(kc_hi - 1)
                # normalize: psum row 64 is the softmax denominator; move it
                # to partition 0 via stream_shuffle (the only legal
                # PSUM-partition mover), reciprocate, then gpsimd-broadcast
                for hh in range(2):
                    pO = pOs[hh]
                    drow = rrp.tile([32, 512], F32, tag="dr")
                    nc.vector.stream_shuffle(
                        drow[:], pO[64:96, :], mask=[0] * 32
                    )
                    rrow = rrp.tile([1, 512], F32, tag="rr")
                    nc.vector.reciprocal_approx_fast(
                        out=rrow[:], in_=drow[0:1, :]
                    )
                    rsb = rbp.tile([64, 512], F32, tag="rb")
                    nc.gpsimd.partition_broadcast(rsb[:], rrow[:], channels=64)
                    if hh == 0:
                        nc.vector.tensor_tensor(
                            out=ONpk[0:64, eb, q0 : q0 + 512],
                            in0=pO[0:64, :],
                            in1=rsb[:],
                            op=ALU.mult,
                        )
                    else:
                        tmp = oddp.tile([64, 512], BF16, tag="odd")
                        nc.vector.tensor_tensor(
                            out=tmp[:], in0=pO[0:64, :], in1=rsb[:], op=ALU.mult
                        )
                        # partition shift 0-63 -> 64-127 via SBUF-to-SBUF DMA
                        nc.sync.dma_start(
                            ONpk[64:128, eb, q0 : q0 + 512], tmp[:]
                        )

            def oproj_piece(s, tn, dn):
                # out[t, d] = ON[e, t].T @ wo[e, d]
                tb = 512 * s + 128 * tn
                acc = auxp.tile([128, 512], F32, tag="aux", name=f"op{s}{tn}{dn}")
                for eb in range(EB):
                    nc.tensor.matmul(
                        acc[:],
                        ONpk[:, eb, tb : tb + 128],
                        wo_sb[:, eb, 512 * dn : 512 * dn + 512],
                        start=(eb == 0),
                        stop=(eb == EB - 1),
                    )
                ob = outsp.tile([128, 512], BF16, tag="ob")
                if (tn + dn) % 2 == 0:
                    nc.vector.tensor_copy(ob[:], acc[:])
                else:
                    nc.scalar.copy(ob[:], acc[:])
                nc.sync.dma_start(
                    outp[tb : tb + 128, 512 * dn : 512 * dn + 512], ob[:]
                )

            proj_stripe(0)
            for s in range(SN):
                inj = None
                if s >= 1:
                    inj = [lambda s=s: qk_piece(s, 0, 1)]
                    inj += [lambda s=s, b=b: v_proj_sub(s, b) for b in range(4)]
                    inj += [lambda s=s: qk_piece(s, 1, 1)]
                att_stripe(0, s, inject=inj)
                att_stripe(1, s)
                if s + 1 < SN:
                    # Q of the next stripe gates its first scores; K/V of the
                    # next stripe are injected inside its attention loop
                    qk_piece(s + 1, 0, 0)
                    qk_piece(s + 1, 1, 0)
                for tn in range(4):
                    for dn in range(2):
                        oproj_piece(s, tn, dn)
    nc.compile()
    return nc


_CACHE = {}
LAST_RESULTS = None


def get_nc():
    if "nc" not in _CACHE:
        _CACHE["nc"] = build_nc()
    return _CACHE["nc"]


def make_in_maps(q, k, v, wq, wk, wv, wo):
    q, k, v, wq, wk, wv, wo = (
        np.asarray(a, dtype=np.float32) for a in (q, k, v, wq, wk, wv, wo)
    )
    scale = 1.0 / math.sqrt(DH)
    xT = [
        (
            np.ascontiguousarray(q[b].T).astype(BF16NP),
            np.ascontiguousarray(k[b].T).astype(BF16NP),
            np.ascontiguousarray(v[b].T).astype(BF16NP),
        )
        for b in range(B)
    ]
    in_maps = []
    for c in range(NCORES):
        b, g = divmod(c, G)
        gs = slice(E * g, E * (g + 1))
        in_maps.append(
            {
                "xqT": xT[b][0],
                "xkT": xT[b][1],
                "xvT": xT[b][2],
                "wqT": np.ascontiguousarray((wq[gs] * scale).T).astype(BF16NP),
                "wkT": np.ascontiguousarray(wk[gs].T).astype(BF16NP),
                "wvT": np.ascontiguousarray(wv[gs].T).astype(BF16NP),
                "woT": np.ascontiguousarray(wo[:, gs].T).astype(BF16NP),
            }
        )
    return in_maps


def kernel(q, k, v, wq, wk, wv, wo):
    global LAST_RESULTS
    nc = get_nc()
    in_maps = make_in_maps(q, k, v, wq, wk, wv, wo)
    res = run_bass_kernel_spmd(nc, in_maps, core_ids=list(range(NCORES)))
    LAST_RESULTS = res
    out = np.zeros((B, T, D), dtype=np.float32)
    for c in range(NCORES):
        out[c // G] += np.asarray(res.results[c]["outp"], dtype=np.float32)
    return out
